# revision 47
# baseline (speedup 1.0000x reference)
"""Clustered Linformer Attention — Trainium2 Bass kernel, 8 NeuronCores.

Strategy: data-parallel over batch (2 batches/core, no collectives).
Math restructuring (verified vs reference to ~7e-7 in f32):
  - mask is all-ones => cluster c holds positions [32c, 32c+32); the per-head
    gather+einsum projections become  k_proj = AE[h]^T @ k_h  with a host-built
    sparse table AE[h] in [S, P] (score scale folded in), same for v with AF.
  - the 3-kernel conv fusion over scores collapses to 5 "tap" matrices M_t in
    [P, P] (t in -2..2):  scores_conv[s] = sum_t  (q[s+t] @ (k_proj^T @ M_t)).
  - v5: the Q GEMM also runs fp8 DoubleRow (x/wq shipped in fp8, wq x16
    with a 1/128 scale on the q copy so both operands and the stored q/8
    dodge e4m3 denormals); q only feeds the already-fp8 scores path, so
    the re-quantization masks most of the upstream quantization error.
  - v4: scores run in fp8(e4m3) DoubleRow: per pair, 3 block-diagonal
    [128, 2-ktile, 128] tap operands (taps paired (-2,-1), (0,+1), (+2,0);
    heads on disjoint partition/col halves) contract against a pair-stacked
    fp8 q tile (slot 0 = raw GEMM layout, slot 1 = +1 shift), so all 5 taps
    for both heads take 3 half-rate passes.  A x8/(1/8) scale folded into
    the host tap tables/wq centers both fp8 operands in e4m3's normal
    range (measured end-to-end rel err 1.56e-2 vs the 2e-2 gate,
    bit-stable across runs).
  - softmax has no max-subtraction (|scores| <~ 1.6, exp is safe in f32);
    Z = sum_c exp is computed by an all-ones block-diag matmul that also
    broadcasts Z to all 128 partitions, so normalization is one DVE op.

Scheduling (v3):
  - dense bias is applied host-side (zero in practice): no k=1 bias matmuls.
  - kt builds each head's tap stack with 2 matmuls (rhs = 3/2 taps wide).
  - weights are DMA'd in dc slices (first kv matmul needs only 128KB); x(0)
    feeds phase A from gpsimd (first quarter split with scalar), x(1) rides
    sync behind the 8 once-fetched persistent ae/af pair tables.
  - qts is per-pair [128, 2, QW] tiles on a 5-deep ring (b1 reuses b0).
  - scores runs one unit ahead of zat (exp latency always covered even when
    dense/GEMM fillers run dry); the last chunk's dense accumulates
    progressively per pair, and the final unit normalizes in 128-col
    quarters so the dc=3 matmuls chase the DVE muls.
  - output is bf16 (upcast on host), copies alternate ACT/DVE, out DMAs
    alternate sync/scalar so gpsimd's queue drains during phase C.
"""
import sys
import numpy as np
import ml_dtypes

sys.path.insert(0, '/opt/trn_rl_repo')

B, S, D = 16, 2048, 512
H, P, C = 8, 64, 32
DEPTH = D // H           # 64
NCORES = 8
BLOC = B // NCORES       # 2 batches per core
NPAIR = H // 2           # 4 head pairs
SCH = 4                  # s-chunks of 512
SCW = S // SCH           # 512
NJ = S // 128            # 16 s-tiles of 128
NDC = D // 128           # 4 contraction chunks
QW = S + 4               # qts width (2 pad front, 2 back)

_CACHE = {}


def _build_nc():
    import concourse.tile as tile
    from concourse import mybir, bacc

    f32 = mybir.dt.float32
    bf16 = mybir.dt.bfloat16
    fp8 = mybir.dt.float8e4
    DR = mybir.MatmulPerfMode.DoubleRow

    nc = bacc.Bacc()
    xT = nc.declare_dram_parameter("xT", [BLOC, D, S], bf16, isOutput=False)
    # fp8 copies for the DoubleRow Q-GEMM (q only feeds the fp8 scores
    # path); wq carries x16 so its entries sit in e4m3's normal range,
    # compensated by a 1/128 scale on the q copy (qts stores q/8).
    xT8 = nc.declare_dram_parameter("xT8", [BLOC, D, S], fp8, isOutput=False)
    wq = nc.declare_dram_parameter("wq", [D, D], fp8, isOutput=False)
    wk = nc.declare_dram_parameter("wk", [D, D], bf16, isOutput=False)
    wv = nc.declare_dram_parameter("wv", [D, D], bf16, isOutput=False)
    dw = nc.declare_dram_parameter("dw", [D, D], bf16, isOutput=False)
    # partition-major cluster tables: ae[pr, p, j, c] = AE_pair[pr][128j+p, c]
    ae = nc.declare_dram_parameter("ae", [NPAIR, 128, NJ, 128], bf16,
                                   isOutput=False)
    af = nc.declare_dram_parameter("af", [NPAIR, 128, NJ, 128], bf16,
                                   isOutput=False)
    # tap matrices, duplicated on both partition halves, in tap order
    # (-2, -1, 0, +1, +2): bdm[p, t, c'] = M_t[p % 64, c'] * 8.
    bdm = nc.declare_dram_parameter("bdm", [128, 5, 64], bf16, isOutput=False)
    onesbd = nc.declare_dram_parameter("onesbd", [128, 128], bf16,
                                       isOutput=False)
    out = nc.declare_dram_parameter("out", [BLOC, S, D], bf16, isOutput=True)

    with tile.TileContext(nc) as tc:
        with tc.tile_pool(name="const", bufs=1) as cpool, \
             tc.tile_pool(name="big", bufs=1) as bigp, \
             tc.tile_pool(name="sm", bufs=4) as smp, \
             tc.tile_pool(name="bd", bufs=8) as bdp, \
             tc.tile_pool(name="ob", bufs=2) as obp, \
             tc.tile_pool(name="psB", bufs=6, space="PSUM") as psB, \
             tc.tile_pool(name="psS", bufs=2, space="PSUM") as psS:

            # ---- constants in SBUF ----
            wq_sb = cpool.tile([128, NDC, D], fp8)
            wk_sb = cpool.tile([128, NDC, D], bf16)
            wv_sb = cpool.tile([128, NDC, D], bf16)
            dw_sb = cpool.tile([128, NDC, D], bf16)
            # dc-sliced weight loads: the first kv matmul only needs
            # wk's dc0 slice (128 KB), not the whole 512 KB tensor, so the
            # ramp starts ~3us earlier.  wk/wv slices interleaved (kv chains
            # consume them in that order), then wq, then dw.
            for dc in range(NDC):
                for t_sb, t_dr in ((wk_sb, wk), (wv_sb, wv)):
                    nc.sync.dma_start(out=t_sb[:, dc, :],
                                      in_=t_dr[128 * dc:128 * (dc + 1), :])
            for t_sb, t_dr in ((wq_sb, wq), (dw_sb, dw)):
                for dc in range(NDC):
                    nc.sync.dma_start(out=t_sb[:, dc, :],
                                      in_=t_dr[128 * dc:128 * (dc + 1), :])
            bdm_sb = cpool.tile([128, 5, 64], bf16)
            nc.sync.dma_start(out=bdm_sb, in_=bdm[:])
            ones_sb = cpool.tile([128, 128], bf16)
            nc.sync.dma_start(out=ones_sb, in_=onesbd[:])

            # cluster tables: 8 persistent SBUF tiles (per pair x ae/af,
            # shared by both batches), each fetched ONCE at start with a
            # single contiguous DMA on the sync queue.
            aexp = {}
            for pr in range(NPAIR):
                for tb, srct in (("ae", ae), ("af", af)):
                    t = cpool.tile([128, NJ, 128], bf16, name=f"aexp_{tb}{pr}")
                    aexp[(tb, pr)] = t
                    nc.sync.dma_start(out=t, in_=srct[pr])

            st = [dict() for _ in range(BLOC)]

            def emit_x_load(b):
                # column-sliced so kv/qt of early s-chunks start ASAP.
                # batch 0 feeds phase A from the gpsimd queue; batch 1 rides
                # the sync queue (behind weights+tables, done well before
                # phase B needs it) so the two batches transfer in parallel.
                s = st[b]
                s["xt"] = [bigp.tile([128, S], bf16, tag="xt", bufs=2 * NDC,
                                     name=f"xt_{b}_{dc}")
                           for dc in range(NDC)]
                s["xt8"] = bigp.tile([128, NDC, S], fp8, tag="xt8", bufs=2,
                                     name=f"xt8_{b}")
                for q in range(SCH):
                    for dc in range(NDC):
                        # batch 0 feeds phase A from gpsimd (first quarter
                        # split with scalar to halve time-to-first-matmul);
                        # batch 1 rides sync behind weights+tables.
                        if b == 0:
                            eng = nc.scalar if (q == 0 and dc >= 2) \
                                else nc.gpsimd
                        else:
                            eng = nc.sync
                        eng.dma_start(
                            out=s["xt"][dc][:, SCW * q:SCW * (q + 1)],
                            in_=xT[b, 128 * dc:128 * (dc + 1),
                                   SCW * q:SCW * (q + 1)])
                    if b == 0 and q == 0:
                        # fp8 x first half right behind the bf16 q0 slices
                        # so qt(0, *, 0) isn't gated on the whole stream
                        for dc in range(NDC):
                            nc.gpsimd.dma_start(
                                out=s["xt8"][:, dc, 0:S // 2],
                                in_=xT8[b, 128 * dc:128 * (dc + 1), 0:S // 2])
                for dc in range(NDC):
                    lo = S // 2 if b == 0 else 0
                    nc.gpsimd.dma_start(
                        out=s["xt8"][:, dc, lo:S],
                        in_=xT8[b, 128 * dc:128 * (dc + 1), lo:S])

            def emit_kv(b, j):
                s = st[b]
                if j == 0:
                    s["knat"] = bigp.tile([128, NJ, D], bf16, tag="knat",
                                          name=f"knat_{b}")
                    s["vnat"] = bigp.tile([128, NJ, D], bf16, tag="vnat",
                                          name=f"vnat_{b}")
                for w_sb, key in ((wk_sb, "knat"), (wv_sb, "vnat")):
                    ps_k = psB.tile([128, D], f32, tag="ps512")
                    for dc in range(NDC):
                        nc.tensor.matmul(
                            ps_k,
                            s["xt"][dc][:, 128 * j:128 * (j + 1)],
                            w_sb[:, dc, :],
                            start=(dc == 0), stop=(dc == NDC - 1))
                    if key == "knat":
                        nc.vector.tensor_copy(out=s[key][:, j, :], in_=ps_k)
                    else:
                        nc.scalar.copy(out=s[key][:, j, :], in_=ps_k)

            def emit_qt(b, pr, n):
                # per-pair PAIR-STACKED fp8 qT tile [128, 2 slots, QW]:
                # partitions 0-63 = q_h0[d, col-2], 64-127 = q_h1[d, col-2]
                # (the raw GEMM output layout -- no staging moves needed).
                # slot 1 is the +1-shifted copy, so DoubleRow ktile pairs
                # (slot0, slot1) at column offset 2g give taps (2g-2, 2g-1).
                s = st[b]
                if n == 0:
                    t = bigp.tile([128, 2, QW], fp8, tag="qtsp", bufs=5,
                                  name=f"qts_{b}_{pr}")
                    s.setdefault("qtsp", {})[pr] = t
                    nc.vector.memset(t[:, 0, 0:2], 0.0)
                    nc.vector.memset(t[:, 0, S + 2:], 0.0)
                    # slot 1's last col is read by group-2 ktile 1 (zero
                    # weights, but fp8 NaN x 0 = NaN) and the +1-shift copy
                    # never writes it: must be a real value.
                    nc.vector.memset(t[:, 1, QW - 1:], 0.0)
                ps_q = psB.tile([128, SCW], f32, tag="ps512")
                for t2 in range(NDC // 2):
                    nc.tensor.matmul(
                        ps_q,
                        wq_sb[:, 2 * t2:2 * t2 + 2, 128 * pr:128 * (pr + 1)],
                        s["xt8"][:, 2 * t2:2 * t2 + 2,
                                 SCW * n:SCW * (n + 1)],
                        start=(t2 == 0), stop=(t2 == NDC // 2 - 1),
                        perf_mode=DR)
                qts = s["qtsp"][pr]
                b0 = 2 + SCW * n       # slot-0 col of s = SCW*n
                # ps_q holds 16*q (wq carries x16); qts stores q/8
                nc.scalar.activation(
                    out=qts[:, 0, b0:b0 + SCW], in_=ps_q,
                    func=mybir.ActivationFunctionType.Copy,
                    scale=1.0 / 128.0)

            def emit_qdup(b, pr):
                # slot 1 = slot 0 shifted one col left (q at +1 shift);
                # a single SBUF->SBUF DMA once the pair's 4 chunks landed.
                s = st[b]
                qts = s["qtsp"][pr]
                nc.gpsimd.dma_start(out=qts[:, 1, 0:QW - 1],
                                    in_=qts[:, 0, 1:QW])

            def emit_proj_fetch(b, pr):
                pass  # tables are persistent; fetched once at start

            def emit_proj(b, pr):
                s = st[b]
                if pr == 0:
                    s["kp"] = bigp.tile([128, NPAIR, 128], bf16, tag="kpbd",
                                        bufs=2, name=f"kp_{b}")
                    s["vp"] = bigp.tile([128, NPAIR, 128], bf16, tag="vpbd",
                                        bufs=2, name=f"vp_{b}")
                    nc.vector.memset(s["vp"], 0.0)
                for a_sb, key, dstk in ((aexp[("ae", pr)], "knat", "kp"),
                                        (aexp[("af", pr)], "vnat", "vp")):
                    ps_p = psS.tile([128, 128], f32, tag="pssmall")
                    for j in range(NJ):
                        nc.tensor.matmul(
                            ps_p,
                            a_sb[:, j, :],
                            st[b][key][:, j, 128 * pr:128 * (pr + 1)],
                            start=(j == 0), stop=(j == NJ - 1))
                    dst = st[b][dstk]
                    if dstk == "kp":
                        # only diag blocks are ever read (per-head lhsT)
                        nc.vector.tensor_copy(out=dst[:, pr, :], in_=ps_p)
                    else:
                        # vp is used as a block-diag [c,d] operand: keep
                        # off-diag zero.
                        nc.vector.tensor_copy(
                            out=dst[0:64, pr, 0:64], in_=ps_p[0:64, 0:64])
                        nc.vector.tensor_copy(
                            out=dst[64:128, pr, 64:128],
                            in_=ps_p[64:128, 64:128])

            def emit_kt(b, pr):
                # per-PAIR block-diagonal DoubleRow tap operand
                # T[p, gk, c]: flat gk = 2g+k covers taps (-2,-1,0,+1,+2)
                # at gk 0..4 (gk 5 = zero pad for the +2 group's ktile 1);
                # head h0 occupies (p<64, c<64), h1 (p>=64, c>=64), with the
                # cross-head blocks zeroed so one matmul serves both heads.
                s = st[b]
                if pr == 0:
                    s["bdts"] = {}
                    s["expt"] = {}
                    s["cw"] = {}
                T = bdp.tile([128, 6, 128], fp8, tag="bdts",
                             name=f"bdts_{b}_{pr}")
                s["bdts"][pr] = T
                nc.vector.memset(T, 0.0)
                for h2 in (0, 1):
                    hb = 64 * h2
                    kp_h = s["kp"][hb:hb + 64, pr, hb:hb + 64]
                    ps_b = psS.tile([128, 5, 64], f32, tag="pssmall")
                    nc.tensor.matmul(ps_b[hb:hb + 64, :, :], kp_h,
                                     bdm_sb[hb:hb + 64, :, :],
                                     start=True, stop=True)
                    nc.scalar.copy(out=T[hb:hb + 64, 0:5, hb:hb + 64],
                                   in_=ps_b[hb:hb + 64, :, :])

            def emit_scores(b, pr, n):
                # 3 K-stacked tap matmuls per head; heads in different PE
                # column groups so consecutive pairs overlap.
                s = st[b]
                qts = s["qtsp"][pr]
                T = s["bdts"][pr]
                ps_sc = psB.tile([128, SCW], f32, tag="ps512")
                base = SCW * n
                # 3 fp8 DoubleRow matmuls, each contracting a tap PAIR for
                # both heads at once (full 128-partition dst).  Group g at
                # column offset 2g pairs taps (2g-2, 2g-1); ktile 1 of the
                # last group multiplies zero weights.
                for g in range(3):
                    nc.tensor.matmul(ps_sc, T[:, 2 * g:2 * g + 2, :],
                                     qts[:, :, base + 2 * g:
                                         base + 2 * g + SCW],
                                     start=(g == 0), stop=(g == 2),
                                     perf_mode=DR)
                expt = smp.tile([128, SCW], bf16, tag="expt", bufs=4)
                nc.scalar.activation(
                    out=expt, in_=ps_sc,
                    func=mybir.ActivationFunctionType.Exp)
                s["expt"][(pr, n)] = expt

            def emit_zat(b, pr, n):
                s = st[b]
                expt = s["expt"].pop((pr, n))
                if pr == 0:
                    s["cw"][n] = bigp.tile([128, NPAIR, SCW], bf16,
                                           tag="cwin", bufs=6,
                                           name=f"cw_{b}_{n}")
                ps_z = psB.tile([128, SCW], f32, tag="ps512")
                nc.tensor.matmul(ps_z, ones_sb, expt, start=True, stop=True)
                ps_at = psB.tile([128, SCW], f32, tag="ps512")
                nc.tensor.matmul(ps_at, s["vp"][:, pr, :], expt,
                                 start=True, stop=True)
                rzb = smp.tile([128, SCW], f32, tag="rzb", bufs=1)
                nc.vector.reciprocal_approx_fast(out=rzb, in_=ps_z)
                nc.vector.tensor_mul(
                    out=s["cw"][n][:, pr, :], in0=ps_at, in1=rzb)

            def emit_dense(b, j):
                s = st[b]
                n, jj = j // 4, j % 4
                cw = s["cw"][n]
                ps_d = psB.tile([128, D], f32, tag="ps512")
                for dc in range(NDC):
                    nc.tensor.matmul(
                        ps_d,
                        cw[:, dc, 128 * jj:128 * (jj + 1)],
                        dw_sb[:, dc, :],
                        start=(dc == 0), stop=(dc == NDC - 1))
                emit_dense_out(b, j, ps_d)

            def emit_dense_out(b, j, ps_d):
                obuf = obp.tile([128, D], bf16, tag="obuf", bufs=4)
                if j % 2:
                    nc.vector.tensor_copy(out=obuf, in_=ps_d)
                else:
                    nc.scalar.copy(out=obuf, in_=ps_d)
                eng = (nc.sync, nc.scalar)[j % 2]
                eng.dma_start(out=out[b, 128 * j:128 * (j + 1), :], in_=obuf)

            def emit_dense_partial(b, j, dcs, ps_d):
                # progressive tail dense: accumulate listed dc chunks of
                # output tile j; finish (copy+DMA) when dc 3 lands.
                s = st[b]
                n, jj = j // 4, j % 4
                cw = s["cw"][n]
                for dc in dcs:
                    nc.tensor.matmul(
                        ps_d,
                        cw[:, dc, 128 * jj:128 * (jj + 1)],
                        dw_sb[:, dc, :],
                        start=(dc == 0), stop=(dc == NDC - 1))
                if dcs[-1] == NDC - 1:
                    emit_dense_out(b, j, ps_d)

            # ================= emission schedule =================
            from collections import deque

            # Phase A: batch-0 GEMMs per x-slice quarter; batch-1 x DMAs
            # queued right behind batch-0's.
            emit_x_load(0)
            emit_x_load(1)
            for q in range(SCH):
                for j in range(4 * q, 4 * q + 4):
                    emit_kv(0, j)
                for pr in range(NPAIR):
                    emit_qt(0, pr, q)
                if q == 2:
                    emit_proj_fetch(0, 0)
                    emit_proj_fetch(0, 1)
            for pr in range(NPAIR):
                emit_qdup(0, pr)
            emit_proj(0, 0)
            emit_proj_fetch(0, 2)
            emit_qt(1, 0, 0)
            emit_kt(0, 0)
            emit_proj(0, 1)
            emit_proj_fetch(0, 3)
            emit_qt(1, 0, 1)
            emit_kt(0, 1)
            emit_proj(0, 2)
            emit_qt(1, 0, 2)
            emit_kt(0, 2)
            emit_proj(0, 3)
            emit_qt(1, 0, 3)
            emit_kt(0, 3)
            emit_qdup(1, 0)

            # Phase B: batch-0 attention with batch-1 GEMM units as fillers.
            qt_units = []
            for pr in range(1, NPAIR):
                qt_units += [(emit_qt, (1, pr, n)) for n in range(SCH)]
                qt_units.append((emit_qdup, (1, pr)))
            fill = deque(
                [(emit_kv, (1, j)) for j in range(8)] +
                [(emit_proj_fetch, (1, 0))] +
                [(emit_kv, (1, j)) for j in range(8, NJ)] +
                [(emit_proj_fetch, (1, 1))] +
                qt_units +
                [(emit_proj, (1, 0)), (emit_proj_fetch, (1, 2)),
                 (emit_kt, (1, 0)),
                 (emit_proj, (1, 1)), (emit_proj_fetch, (1, 3)),
                 (emit_kt, (1, 1)),
                 (emit_proj, (1, 2)), (emit_kt, (1, 2)),
                 (emit_proj, (1, 3)), (emit_kt, (1, 3))])

            def popf():
                # emit filler units until one with PE work was emitted
                while fill:
                    f, a = fill.popleft()
                    f(*a)
                    if f not in (emit_proj_fetch, emit_qdup):
                        break

            # scores runs one unit ahead of zat so the ACT exp latency is
            # always covered by the next unit's matmuls even with no fillers.
            prev = None
            for pr in range(NPAIR):
                for n in range(SCH):
                    emit_scores(0, pr, n)
                    popf()
                    if prev is not None:
                        emit_zat(0, *prev)
                        popf()
                    prev = (pr, n)
            emit_zat(0, *prev)
            while fill:
                popf()

            # Phase C: batch-1 attention (scores pipelined one ahead of zat);
            # fillers are batch-0 dense then batch-1 dense as chunks complete.
            # The last chunk's dense is accumulated progressively per pair so
            # only the dc=3 matmuls remain after the final zat.
            fill = deque([(emit_dense, (0, j)) for j in range(NJ)])
            prev = None
            ps_tail = None
            for n in range(SCH):
                for pr in range(NPAIR):
                    emit_scores(1, pr, n)
                    popf()
                    if prev is not None:
                        emit_zat(1, *prev)
                        popf()
                        if prev[1] == SCH - 1 and prev[0] == 2:
                            # pairs 0-2 of the last chunk are done: run their
                            # dense contributions now (dc = pair index).
                            ps_tail = [psB.tile([128, D], f32, tag="ps512",
                                                name=f"ps_tail_{jj}")
                                       for jj in range(4)]
                            for jj, ps_d in enumerate(ps_tail):
                                emit_dense_partial(1, 12 + jj, (0, 1, 2), ps_d)
                    prev = (pr, n)
                if n < SCH - 1:
                    for j in range(4 * n, 4 * n + 4):
                        fill.append((emit_dense, (1, j)))
            # final unit: quarter-split normalize so each dc=3 dense matmul
            # starts right after its 128-col quarter of cw is normalized.
            s = st[1]
            pr, n = prev
            expt = s["expt"].pop((pr, n))
            ps_z = psB.tile([128, SCW], f32, tag="ps512")
            nc.tensor.matmul(ps_z, ones_sb, expt, start=True, stop=True)
            ps_at = psB.tile([128, SCW], f32, tag="ps512")
            nc.tensor.matmul(ps_at, s["vp"][:, pr, :], expt,
                             start=True, stop=True)
            rzb = smp.tile([128, SCW], f32, tag="rzb", bufs=1)
            cwl = s["cw"][n]
            for qq in range(4):
                sl = slice(128 * qq, 128 * (qq + 1))
                nc.vector.reciprocal_approx_fast(out=rzb[:, sl],
                                                 in_=ps_z[:, sl])
                nc.vector.tensor_mul(
                    out=cwl[:, pr, sl], in0=ps_at[:, sl], in1=rzb[:, sl])
                emit_dense_partial(1, 12 + qq, (3,), ps_tail[qq])
            while fill:
                popf()

    nc.finalize()
    return nc


def _prep_inputs(x, mask, wq, wk, wv, EW, FW, conv_w1, conv_w3, conv_w5, conv_b,
                 dense_w, dense_b, cluster_table):
    """Host-side restructuring -> per-core input maps."""
    bf = ml_dtypes.bfloat16
    x = np.ascontiguousarray(np.asarray(x, np.float32))
    mask = np.asarray(mask)
    counts = np.clip(mask.astype(np.int64).sum(1), 1, S)
    pos = np.asarray(cluster_table)[counts - 1]          # [B, P, C]
    if not (pos == pos[0]).all():
        raise NotImplementedError("per-batch cluster tables not supported")
    p0 = pos[0]                                          # [P, C]

    scale = 1.0 / np.sqrt(np.float32(DEPTH))
    s_idx = p0.ravel()
    c_idx = np.repeat(np.arange(P), C)

    def build_table(W, sc):
        A = np.zeros((H, S + 1, P), np.float32)
        np.add.at(A, (np.arange(H)[:, None], s_idx[None, :], c_idx[None, :]),
                  np.asarray(W, np.float32).reshape(H, P * C) * sc)
        return np.ascontiguousarray(A[:, :S, :])

    AE = build_table(EW, scale)
    AF = build_table(FW, 1.0)
    # pack adjacent heads side by side: [NPAIR, S, 128]
    AE = np.ascontiguousarray(
        AE.reshape(NPAIR, 2, S, P).transpose(0, 2, 1, 3).reshape(NPAIR, S, 128))
    AF = np.ascontiguousarray(
        AF.reshape(NPAIR, 2, S, P).transpose(0, 2, 1, 3).reshape(NPAIR, S, 128))
    # partition-major for fast DMA: [NPAIR, 128, NJ, 128]
    AE = np.ascontiguousarray(
        AE.reshape(NPAIR, NJ, 128, 128).transpose(0, 2, 1, 3))
    AF = np.ascontiguousarray(
        AF.reshape(NPAIR, NJ, 128, 128).transpose(0, 2, 1, 3))



    # conv -> 5 tap matrices (per-head [P, P], duplicated on both halves)
    wp = np.arange(P)[:, None]
    jj = np.arange(P)[None, :]
    ii = wp - jj + 31
    valid = (ii >= 0) & (ii < P)
    ii = np.clip(ii, 0, P - 1)
    M = {t: np.zeros((P, P), np.float32) for t in range(-2, 3)}
    for cw, hk in ((conv_w1, 1), (conv_w3, 3), (conv_w5, 5)):
        cw = np.asarray(cw, np.float32)
        pad = (hk - 1) // 2
        for dy in range(hk):
            filt = cw[dy, :, 0, 0]
            M[dy - pad] += np.where(valid, filt[ii], 0.0) / 3.0
    # x8 folded here (and 1/8 into wq): centers the fp8 scores operands
    # in e4m3's normal range; the product is unchanged.
    BDM5 = np.zeros((128, 5, P), np.float32)
    for k, t in enumerate((-2, -1, 0, 1, 2)):
        BDM5[:64, k, :] = M[t] * 8.0
        BDM5[64:, k, :] = M[t] * 8.0
    bbar = float(np.asarray(conv_b, np.float32).mean())
    if abs(bbar) > 1e-30:
        raise NotImplementedError("nonzero conv bias not folded")

    ones_bd = np.zeros((128, 128), np.float32)
    ones_bd[:64, :64] = 1.0
    ones_bd[64:, 64:] = 1.0

    # shard + transpose x
    xsh = x.reshape(NCORES, BLOC, S, D)
    in_maps = []
    f8 = ml_dtypes.float8_e4m3
    shared = dict(
        wq=(np.asarray(wq, np.float32) * 16.0).astype(f8),
        wk=np.asarray(wk, np.float32).astype(bf),
        wv=np.asarray(wv, np.float32).astype(bf),
        dw=np.asarray(dense_w, np.float32).astype(bf),
        ae=AE.astype(bf), af=AF.astype(bf),
        bdm=BDM5.astype(bf),
        onesbd=ones_bd.astype(bf),
    )
    for c in range(NCORES):
        m = dict(shared)
        xc = np.ascontiguousarray(xsh[c].transpose(0, 2, 1))
        m["xT"] = xc.astype(bf)
        m["xT8"] = xc.astype(f8)
        in_maps.append(m)
    return in_maps


def _run(in_maps, trace=False, tmpdir=None):
    from concourse.bass_utils import run_bass_kernel_spmd
    if "nc" not in _CACHE:
        _CACHE["nc"] = _build_nc()
    kw = {}
    if trace:
        _install_ntff_hook()
        kw = dict(trace=True, tmpdir=tmpdir)
    return run_bass_kernel_spmd(_CACHE["nc"], in_maps,
                                core_ids=list(range(NCORES)), **kw)


def _install_ntff_hook():
    import types, importlib.util as ilu
    if "antenv.axon_hooks" in sys.modules:
        return
    spec = ilu.spec_from_file_location(
        "trn_boot_mod", "/root/.axon_site/trn_agent_boot/trn_boot.py")
    tb = ilu.module_from_spec(spec)
    spec.loader.exec_module(tb)
    hook = tb._ntff_profile_via_ctypes("/opt/axon/libaxon_pjrt.so")
    mod = types.ModuleType("antenv.axon_hooks")
    mod.get_axon_ntff_profile_hook = lambda: hook
    import antenv  # noqa: F401
    sys.modules["antenv.axon_hooks"] = mod


def kernel(**inputs) -> np.ndarray:
    in_maps = _prep_inputs(**inputs)
    r = _run(in_maps)
    out = np.concatenate([np.asarray(r.results[c]["out"], np.float32)
                          for c in range(NCORES)], axis=0)
    db = np.asarray(inputs["dense_b"], np.float32)
    if np.any(db):  # dense bias applied host-side (zero in practice)
        out = out + db
    return out

